# revision 37
# baseline (speedup 1.0000x reference)
"""Trainium2 Bass kernel for nn_BasicBlock (binarized 3x3 conv + BN + ReLU).

Reference computation (NHWC, f32):
    a   = ste_sign(x + bias1)            # +-1, sign(0)=+1
    qk  = ste_sign(kernel)               # +-1
    y   = conv2d(a, qk, SAME, stride 1)  # (32,56,56,256)
    y   = (y - mean) * rsqrt(var+eps) + beta
    out = relu(y + bias2)

Strategy (v2, fp8 DoubleRow):
  - Data-parallel over batch: 8 cores x 4 images, no collectives.
  - Operands are exactly +-1, exact in fp8e4; fp32 PSUM accumulation keeps
    integer conv sums (|y| <= 2304) bit-exact. DoubleRow packs both
    128-channel Cin halves into one matmul at 2 MACs/cell/cycle.
  - Per core pipeline, per image:
      load 8-row groups [112 part, 4 px, 256ch] (4KB/partition descriptors)
      -> PE transpose f32 -> ScalarE Sign(x+bias1) -> fp8 +-1 into a
      zero-padded 58-wide channel-major image buffer
      -> conv: weights-stationary fp8 DoubleRow matmuls, psum [co,464px]
         (8 output rows per group, 9 taps accumulated, 2 Cout tiles)
      -> VectorE BN affine (y*s + t, per-partition scale/shift)
      -> PE transpose back to [px, co] per 2-row tile
      -> VectorE fused relu + PSUM evacuation into a per-image staging
         buffer -> 2 large NHWC stores per image via GPSIMD (SWDGE).
  - Host precomputes constants only: sign(kernel) in fp8 DoubleRow layout,
    folded BN scale/shift, and a 1-ulp nudge of x where fl(x+bias1)==0 so
    device Sign (sign(0)=0) matches ste_sign (sign(0)=+1).
"""

import numpy as np
import ml_dtypes

import concourse.bass as bass
import concourse.mybir as mybir
import concourse.tile as tile
from concourse import bacc
from concourse.bass_utils import run_bass_kernel_spmd
from concourse.masks import make_identity
from concourse.tile_rust import add_dep_helper

# Problem shape (hardcoded per contract).
B, H, W, CIN, COUT = 32, 56, 56, 256, 256
N_CORES = 8
IMG = B // N_CORES          # images per core
EPS = 1e-3

P = 128
WPAD = 58                   # padded row width (56 + 2)
ROWS = 59                   # 1 top pad + 56 real + 1 bottom pad + slack
AFREE = 3424                # ROWS*WPAD=3422 padded to %16 for DoubleRow APs
RP = H // 2                 # 28 row-pairs per image
G8 = H // 8                 # 7 eight-row conv groups per image
G4 = H // 4                 # 14 four-row input groups per image
NPX = 8 * WPAD              # 464 psum pixels per conv group

F32 = mybir.dt.float32
FP8 = mybir.dt.float8e4

AluOp = mybir.AluOpType


def _build_program():
    nc = bacc.Bacc(
        "TRN2",
        target_bir_lowering=False,
        debug=False,
        enable_asserts=False,
        num_devices=N_CORES,
    )

    x_ap = nc.dram_tensor("x", (IMG, H, W, CIN), F32, kind="ExternalInput").ap()
    w_ap = nc.dram_tensor("wq", (P, 9, 2, 2 * P), FP8, kind="ExternalInput").ap()
    b1_ap = nc.dram_tensor("b1", (CIN,), F32, kind="ExternalInput").ap()
    s_ap = nc.dram_tensor("s", (2, P), F32, kind="ExternalInput").ap()
    t_ap = nc.dram_tensor("t", (2, P), F32, kind="ExternalInput").ap()
    out_ap = nc.dram_tensor("out", (IMG, H, W, COUT), F32, kind="ExternalOutput").ap()

    x_flat = x_ap.rearrange("b h w c -> b (h w) c")

    with tile.TileContext(nc) as tc:
        with (
            tc.tile_pool(name="const", bufs=1) as const_pool,
            tc.tile_pool(name="xin", bufs=4) as x_pool,
            tc.tile_pool(name="ybn", bufs=16) as y_pool,
            tc.tile_pool(name="pst", bufs=2, space="PSUM") as pst_pool,
            tc.tile_pool(name="pso", bufs=3, space="PSUM") as pso_pool,
            tc.tile_pool(name="psu", bufs=3, space="PSUM") as psu_pool,
        ):
            # Identity + activation-pad memsets first so the first image's
            # transposes/signs aren't queued behind the large constant DMAs.
            ident = const_pool.tile([P, P], F32)
            make_identity(nc, ident[:])
            identb = const_pool.tile([P, P], mybir.dt.bfloat16)
            make_identity(nc, identb[:])
            acts = [
                const_pool.tile([P, 2, AFREE], FP8, name=f"act{i}") for i in range(2)
            ]
            nc.gpsimd.memset(acts[0][:], 0.0)
            nc.gpsimd.memset(acts[1][:], 0.0)

            b1_sb = const_pool.tile([P, CIN], F32)
            nc.sync.dma_start(b1_sb[:], b1_ap[None, :].to_broadcast((P, CIN)))
            # w/s/t are first needed by image 0's conv phase; issued inside
            # the b==0 iteration, after its input loads.
            w_sb = const_pool.tile([P, 9, 2, 2 * P], FP8)
            s_sb = const_pool.tile([P, 2], F32)
            t_sb = const_pool.tile([P, 2], F32)

            ubig = [
                const_pool.tile([P, RP, COUT], F32, name=f"ubig{i}") for i in range(2)
            ]

            for b in range(IMG):
                slot = b % 2
                act = acts[slot]
                ub = ubig[slot]

                # ---- load + binarize + transpose (4-row groups) ----
                # a = (fl(x + bias1) >= 0) - 0.5  in {-0.5, +0.5} bf16, exact
                # (the conv then yields y/2; the x2 is folded into BN scale).
                for g in range(G4):
                    xt = x_pool.tile([112, 2, CIN], F32)
                    nc.sync.dma_start(
                        xt[:],
                        x_flat[b, 224 * g : 224 * (g + 1), :].rearrange(
                            "(p j) c -> p j c", p=112
                        ),
                    )
                    u = x_pool.tile([112, 2, CIN], F32, name="u", tag="u")
                    nc.vector.tensor_tensor(
                        u[:], xt[:],
                        b1_sb[:112, None, :].to_broadcast((112, 2, CIN)),
                        AluOp.add,
                    )
                    a8 = x_pool.tile(
                        [112, 2, CIN], mybir.dt.bfloat16, name="a8", tag="a8"
                    )
                    nc.vector.tensor_scalar(
                        a8[:], u[:], 0.0, 0.5, op0=AluOp.is_ge, op1=AluOp.subtract
                    )
                    for j in range(2):
                        # both ci halves transposed into one psum tile, then
                        # one strided fp8 copy into the padded image buffer.
                        pt2 = pst_pool.tile([P, 224], mybir.dt.bfloat16)
                        for ci in range(2):
                            nc.tensor.matmul(
                                pt2[:, ci * 112 : (ci + 1) * 112],
                                a8[:, j, ci * P : (ci + 1) * P],
                                identb[:112, :112],
                                is_transpose=True,
                                start=(ci == 0),
                                stop=(ci == 1),
                            )
                        # px = 2p + j -> row r = px//56 (4 rows), col 2q+j.
                        # dest: padded rows 4g+1..4g+4, cols 2+j step 2.
                        base = (4 * g + 1) * WPAD + 2 + j
                        dest = (
                            act[:, :, base : base + 4 * WPAD]
                            .rearrange("p c (r w) -> p c r w", w=WPAD)[:, :, :, 0:56]
                            .rearrange("p c r (q x) -> p c r q x", x=2)[:, :, :, :, 0]
                        )
                        nc.scalar.copy(
                            dest,
                            pt2.rearrange("p (c r q) -> p c r q", c=2, q=28),
                        )

                if b == 0:
                    nc.sync.dma_start(w_sb[:], w_ap)
                    nc.sync.dma_start(s_sb[:], s_ap.rearrange("t p -> p t"))
                    nc.sync.dma_start(t_sb[:], t_ap.rearrange("t p -> p t"))

                # ---- conv (fp8 SwInterleave, weights stationary) + BN affine ----
                y_tiles = {}
                for co in range(2):
                    for m in range(G8):
                        pm = pso_pool.tile([P, NPX], F32, name="pm", tag="pm")
                        for tap in range(9):
                            dh, dw = tap // 3, tap % 3
                            rbase = (8 * m + dh) * WPAD + dw
                            nc.tensor.matmul(
                                pm[:],
                                w_sb[:, tap, co],
                                act[:, :, rbase : rbase + NPX],
                                start=(tap == 0),
                                stop=(tap == 8),
                                perf_mode=mybir.MatmulPerfMode.DoubleRowSwInterleave,
                            )
                        y = y_pool.tile([P, NPX], F32, name="y", tag="y")
                        # y = conv * scale + shift   (per-partition co consts)
                        nc.scalar.activation(
                            y[:], pm[:],
                            mybir.ActivationFunctionType.Identity,
                            bias=t_sb[:, co : co + 1],
                            scale=s_sb[:, co : co + 1],
                        )
                        y_tiles[(co, m)] = y

                # ---- transpose back to [px, co], fused relu, stage, store ----
                ev = out_ap[b].rearrange("(k two) w c -> w two k c", two=2)
                for m in range(G8):
                    for r in range(4):
                        k = 4 * m + r
                        pu = psu_pool.tile([116, COUT], F32)
                        for co in range(2):
                            nc.tensor.matmul(
                                pu[:, co * P : (co + 1) * P],
                                y_tiles[(co, m)][:, 116 * r : 116 * r + 116],
                                ident[:, :P],
                                is_transpose=True,
                                start=(co == 0),
                                stop=(co == 1),
                            )
                        # partitions: 0 pad | 1..56 row 2k | 57,58 pad |
                        # 59..114 row 2k+1 | 115 pad
                        nc.vector.tensor_scalar(
                            ub[:116, k, :], pu[:], 0.0, None, op0=AluOp.max
                        )
                    # store the 8 finished rows (4 row-pairs) of this group,
                    # split across the two DGE paths so issue overlaps.
                    ksl = slice(4 * m, 4 * m + 4)
                    nc.gpsimd.dma_start(ev[:, 0, ksl], ub[1:57, ksl])
                    nc.gpsimd.dma_start(ev[:, 1, ksl], ub[59:115, ksl])

    nc.compile()
    return nc


_NC_CACHE = None


def _get_nc():
    global _NC_CACHE
    if _NC_CACHE is None:
        _NC_CACHE = _build_program()
    return _NC_CACHE


def _prep_inputs(x, bias1, kernel, bn_beta, bn_mean, bn_var, bias2):
    x = np.asarray(x, dtype=np.float32)
    bias1 = np.asarray(bias1, dtype=np.float32)
    kernel = np.asarray(kernel, dtype=np.float32)
    bn_beta = np.asarray(bn_beta, dtype=np.float32)
    bn_mean = np.asarray(bn_mean, dtype=np.float32)
    bn_var = np.asarray(bn_var, dtype=np.float32)
    bias2 = np.asarray(bias2, dtype=np.float32).reshape(-1)

    # Weights: ste_sign with sign(0)=+1, exact in fp8e4.
    # DoubleRowSwInterleave stationary layout per (tap, co_t):
    # [A127 B127 A126 B126 ... A0 B0] where A/B are the ci halves
    # (ci = o*128 + ki, matching the act buffer's [ki, ci_t, px] pairing)
    # and columns are stored co-reversed.
    wq = np.where(kernel >= 0, np.float32(1.0), np.float32(-1.0))
    wq = wq.reshape(9, 2, P, 2, P)[..., ::-1]       # [tap, o, ki, co_t, m]
    wq = wq.transpose(2, 0, 3, 4, 1)                # [ki, tap, co_t, m, o]
    wq = np.ascontiguousarray(wq).reshape(P, 9, 2, 2 * P)
    wq = wq.astype(ml_dtypes.float8_e4m3)

    s = (1.0 / np.sqrt(bn_var + np.float32(EPS))).astype(np.float32)
    t = (bn_beta - bn_mean * s + bias2).astype(np.float32)
    # activations are encoded as sign/2, so the conv yields y/2: scale by 2s.
    s2 = (2.0 * s).astype(np.float32)

    in_maps = []
    for c in range(N_CORES):
        in_maps.append(
            {
                "x": np.ascontiguousarray(x[c * IMG : (c + 1) * IMG]),
                "wq": wq,
                "b1": np.ascontiguousarray(bias1.reshape(-1)),
                "s": np.ascontiguousarray(s2.reshape(2, P)),
                "t": np.ascontiguousarray(t.reshape(2, P)),
            }
        )
    return in_maps


def _ensure_ntff_hook():
    """This container ships the NTFF profiling machinery but not the
    ``antenv.axon_hooks`` shim module bass_utils imports it through;
    synthesize it so trace=True can capture HW exec times."""
    import sys
    import types

    if "antenv.axon_hooks" in sys.modules:
        return
    import antenv
    from trn_agent_boot.trn_boot import _ntff_profile_via_ctypes

    hook = _ntff_profile_via_ctypes("/opt/axon/libaxon_pjrt.so")
    mod = types.ModuleType("antenv.axon_hooks")
    mod.get_axon_ntff_profile_hook = lambda: hook
    mod.set_axon_ntff_profile_hook = lambda h: None
    sys.modules["antenv.axon_hooks"] = mod
    antenv.axon_hooks = mod


def run(inputs: dict, trace: bool = False):
    """Run the SPMD kernel. Returns (out, exec_time_ns or None)."""
    nc = _get_nc()
    in_maps = _prep_inputs(**inputs)
    if trace:
        try:
            _ensure_ntff_hook()
        except Exception as e:  # degrade to untraced run
            print(f"ntff hook unavailable: {e}")
    res = run_bass_kernel_spmd(
        nc, in_maps, core_ids=list(range(N_CORES)), trace=trace
    )
    out = np.concatenate([r["out"] for r in res.results], axis=0)
    return out, res.exec_time_ns


def kernel(**inputs) -> np.ndarray:
    out, _ = run(inputs, trace=False)
    return out


# revision 39
# speedup vs baseline: 1.1386x; 1.1386x over previous
"""Trainium2 Bass kernel for nn_BasicBlock (binarized 3x3 conv + BN + ReLU).

Reference computation (NHWC, f32):
    a   = ste_sign(x + bias1)            # +-1, sign(0)=+1
    qk  = ste_sign(kernel)               # +-1
    y   = conv2d(a, qk, SAME, stride 1)  # (32,56,56,256)
    y   = (y - mean) * rsqrt(var+eps) + beta
    out = relu(y + bias2)

Strategy (v2, fp8 DoubleRow):
  - Data-parallel over batch: 8 cores x 4 images, no collectives.
  - Operands are exactly +-1, exact in fp8e4; fp32 PSUM accumulation keeps
    integer conv sums (|y| <= 2304) bit-exact. DoubleRow packs both
    128-channel Cin halves into one matmul at 2 MACs/cell/cycle.
  - Per core pipeline, per image:
      load 8-row groups [112 part, 4 px, 256ch] (4KB/partition descriptors)
      -> PE transpose f32 -> ScalarE Sign(x+bias1) -> fp8 +-1 into a
      zero-padded 58-wide channel-major image buffer
      -> conv: weights-stationary fp8 DoubleRow matmuls, psum [co,464px]
         (8 output rows per group, 9 taps accumulated, 2 Cout tiles)
      -> VectorE BN affine (y*s + t, per-partition scale/shift)
      -> PE transpose back to [px, co] per 2-row tile
      -> VectorE fused relu + PSUM evacuation into a per-image staging
         buffer -> 2 large NHWC stores per image via GPSIMD (SWDGE).
  - Host precomputes constants only: sign(kernel) in fp8 DoubleRow layout,
    folded BN scale/shift, and a 1-ulp nudge of x where fl(x+bias1)==0 so
    device Sign (sign(0)=0) matches ste_sign (sign(0)=+1).
"""

import numpy as np
import ml_dtypes

import concourse.bass as bass
import concourse.mybir as mybir
import concourse.tile as tile
from concourse import bacc
from concourse.bass_utils import run_bass_kernel_spmd
from concourse.masks import make_identity
from concourse.tile_rust import add_dep_helper

# Problem shape (hardcoded per contract).
B, H, W, CIN, COUT = 32, 56, 56, 256, 256
N_CORES = 8
IMG = B // N_CORES          # images per core
EPS = 1e-3

P = 128
WPAD = 58                   # padded row width (56 + 2)
ROWS = 59                   # 1 top pad + 56 real + 1 bottom pad + slack
AFREE = 3424                # ROWS*WPAD=3422 padded to %16 for DoubleRow APs
RP = H // 2                 # 28 row-pairs per image
G8 = H // 8                 # 7 eight-row conv groups per image
G4 = H // 4                 # 14 four-row input groups per image
NPX = 8 * WPAD              # 464 psum pixels per conv group

F32 = mybir.dt.float32
FP8 = mybir.dt.float8e4

AluOp = mybir.AluOpType


def _build_program():
    nc = bacc.Bacc(
        "TRN2",
        target_bir_lowering=False,
        debug=False,
        enable_asserts=False,
        num_devices=N_CORES,
    )

    x_ap = nc.dram_tensor("x", (IMG, H, W, CIN), F32, kind="ExternalInput").ap()
    w_ap = nc.dram_tensor("wq", (P, 9, 2, 2 * P), FP8, kind="ExternalInput").ap()
    b1_ap = nc.dram_tensor("b1", (CIN,), F32, kind="ExternalInput").ap()
    s_ap = nc.dram_tensor("s", (2, P), F32, kind="ExternalInput").ap()
    t_ap = nc.dram_tensor("t", (2, P), F32, kind="ExternalInput").ap()
    out_ap = nc.dram_tensor("out", (IMG, H, W, COUT), F32, kind="ExternalOutput").ap()

    x_flat = x_ap.rearrange("b h w c -> b (h w) c")

    with tile.TileContext(nc) as tc:
        with (
            tc.tile_pool(name="const", bufs=1) as const_pool,
            tc.tile_pool(name="xin", bufs=4) as x_pool,
            tc.tile_pool(name="ybn", bufs=16) as y_pool,
            tc.tile_pool(name="pst", bufs=2, space="PSUM") as pst_pool,
            tc.tile_pool(name="pso", bufs=3, space="PSUM") as pso_pool,
            tc.tile_pool(name="psu", bufs=3, space="PSUM") as psu_pool,
        ):
            # Identity + activation-pad memsets first so the first image's
            # transposes/signs aren't queued behind the large constant DMAs.
            ident = const_pool.tile([P, P], F32)
            make_identity(nc, ident[:])
            identb = const_pool.tile([P, P], mybir.dt.bfloat16)
            make_identity(nc, identb[:])
            acts = [
                const_pool.tile([P, 2, AFREE], FP8, name=f"act{i}") for i in range(2)
            ]
            nc.gpsimd.memset(acts[0][:], 0.0)
            nc.gpsimd.memset(acts[1][:], 0.0)

            b1_sb = const_pool.tile([P, CIN], F32)
            nc.sync.dma_start(b1_sb[:], b1_ap[None, :].to_broadcast((P, CIN)))
            # w/s/t are first needed by image 0's conv phase; issued inside
            # the b==0 iteration, after its input loads.
            w_sb = const_pool.tile([P, 9, 2, 2 * P], FP8)
            s_sb = const_pool.tile([P, 2], F32)
            t_sb = const_pool.tile([P, 2], F32)

            ubig = [
                const_pool.tile([P, RP, COUT], F32, name=f"ubig{i}") for i in range(2)
            ]

            def phase_a(b):
                # ---- load + binarize + transpose (4-row groups) ----
                # a = (fl(x + bias1) >= 0) - 0.5  in {-0.5, +0.5} bf16, exact
                # (the conv then yields y/2; the x2 is folded into BN scale).
                act = acts[b % 2]
                for g in range(G4):
                    xt = x_pool.tile([112, 2, CIN], F32, name="xt", tag="xt")
                    nc.sync.dma_start(
                        xt[:],
                        x_flat[b, 224 * g : 224 * (g + 1), :].rearrange(
                            "(p j) c -> p j c", p=112
                        ),
                    )
                    u = x_pool.tile([112, 2, CIN], F32, name="u", tag="u")
                    nc.vector.tensor_tensor(
                        u[:], xt[:],
                        b1_sb[:112, None, :].to_broadcast((112, 2, CIN)),
                        AluOp.add,
                    )
                    a8 = x_pool.tile(
                        [112, 2, CIN], mybir.dt.bfloat16, name="a8", tag="a8"
                    )
                    nc.vector.tensor_scalar(
                        a8[:], u[:], 0.0, 0.5, op0=AluOp.is_ge, op1=AluOp.subtract
                    )
                    for j in range(2):
                        # both ci halves transposed into one psum tile, then
                        # one strided fp8 copy into the padded image buffer.
                        pt2 = pst_pool.tile([P, 224], mybir.dt.bfloat16)
                        for ci in range(2):
                            nc.tensor.matmul(
                                pt2[:, ci * 112 : (ci + 1) * 112],
                                a8[:, j, ci * P : (ci + 1) * P],
                                identb[:112, :112],
                                is_transpose=True,
                                start=(ci == 0),
                                stop=(ci == 1),
                            )
                        # px = 2p + j -> row r = px//56 (4 rows), col 2q+j.
                        # dest: padded rows 4g+1..4g+4, cols 2+j step 2.
                        base = (4 * g + 1) * WPAD + 2 + j
                        dest = (
                            act[:, :, base : base + 4 * WPAD]
                            .rearrange("p c (r w) -> p c r w", w=WPAD)[:, :, :, 0:56]
                            .rearrange("p c r (q x) -> p c r q x", x=2)[:, :, :, :, 0]
                        )
                        nc.scalar.copy(
                            dest,
                            pt2.rearrange("p (c r q) -> p c r q", c=2, q=28),
                        )

            phase_a(0)
            nc.sync.dma_start(w_sb[:], w_ap)
            nc.sync.dma_start(s_sb[:], s_ap.rearrange("t p -> p t"))
            nc.sync.dma_start(t_sb[:], t_ap.rearrange("t p -> p t"))

            for b in range(IMG):
                slot = b % 2
                act = acts[slot]
                ub = ubig[slot]

                # ---- conv (fp8 SwInterleave, weights stationary) + BN affine ----
                y_tiles = {}
                for co in range(2):
                    for m in range(G8):
                        pm = pso_pool.tile([P, NPX], F32, name="pm", tag="pm")
                        for tap in range(9):
                            dh, dw = tap // 3, tap % 3
                            rbase = (8 * m + dh) * WPAD + dw
                            nc.tensor.matmul(
                                pm[:],
                                w_sb[:, tap, co],
                                act[:, :, rbase : rbase + NPX],
                                start=(tap == 0),
                                stop=(tap == 8),
                                perf_mode=mybir.MatmulPerfMode.DoubleRowSwInterleave,
                            )
                        y = y_pool.tile([P, NPX], F32, name="y", tag="y")
                        # y = conv * scale + shift   (per-partition co consts)
                        nc.scalar.activation(
                            y[:], pm[:],
                            mybir.ActivationFunctionType.Identity,
                            bias=t_sb[:, co : co + 1],
                            scale=s_sb[:, co : co + 1],
                        )
                        y_tiles[(co, m)] = y

                # next image's input pipeline goes ahead of this image's
                # output phase so DVE/PE aren't blocked at the boundary.
                if b + 1 < IMG:
                    phase_a(b + 1)

                # ---- transpose back to [px, co], fused relu, stage, store ----
                ev = out_ap[b].rearrange("(k two) w c -> w two k c", two=2)
                for m in range(G8):
                    for r in range(4):
                        k = 4 * m + r
                        pu = psu_pool.tile([116, COUT], F32)
                        for co in range(2):
                            nc.tensor.matmul(
                                pu[:, co * P : (co + 1) * P],
                                y_tiles[(co, m)][:, 116 * r : 116 * r + 116],
                                ident[:, :P],
                                is_transpose=True,
                                start=(co == 0),
                                stop=(co == 1),
                            )
                        # partitions: 0 pad | 1..56 row 2k | 57,58 pad |
                        # 59..114 row 2k+1 | 115 pad
                        nc.vector.tensor_scalar(
                            ub[:116, k, :], pu[:], 0.0, None, op0=AluOp.max
                        )
                    # store the 8 finished rows (4 row-pairs) of this group,
                    # split across the two DGE paths so issue overlaps.
                    ksl = slice(4 * m, 4 * m + 4)
                    nc.gpsimd.dma_start(ev[:, 0, ksl], ub[1:57, ksl])
                    nc.gpsimd.dma_start(ev[:, 1, ksl], ub[59:115, ksl])

    nc.compile()
    return nc


_NC_CACHE = None


def _get_nc():
    global _NC_CACHE
    if _NC_CACHE is None:
        _NC_CACHE = _build_program()
    return _NC_CACHE


def _prep_inputs(x, bias1, kernel, bn_beta, bn_mean, bn_var, bias2):
    x = np.asarray(x, dtype=np.float32)
    bias1 = np.asarray(bias1, dtype=np.float32)
    kernel = np.asarray(kernel, dtype=np.float32)
    bn_beta = np.asarray(bn_beta, dtype=np.float32)
    bn_mean = np.asarray(bn_mean, dtype=np.float32)
    bn_var = np.asarray(bn_var, dtype=np.float32)
    bias2 = np.asarray(bias2, dtype=np.float32).reshape(-1)

    # Weights: ste_sign with sign(0)=+1, exact in fp8e4.
    # DoubleRowSwInterleave stationary layout per (tap, co_t):
    # [A127 B127 A126 B126 ... A0 B0] where A/B are the ci halves
    # (ci = o*128 + ki, matching the act buffer's [ki, ci_t, px] pairing)
    # and columns are stored co-reversed.
    wq = np.where(kernel >= 0, np.float32(1.0), np.float32(-1.0))
    wq = wq.reshape(9, 2, P, 2, P)[..., ::-1]       # [tap, o, ki, co_t, m]
    wq = wq.transpose(2, 0, 3, 4, 1)                # [ki, tap, co_t, m, o]
    wq = np.ascontiguousarray(wq).reshape(P, 9, 2, 2 * P)
    wq = wq.astype(ml_dtypes.float8_e4m3)

    s = (1.0 / np.sqrt(bn_var + np.float32(EPS))).astype(np.float32)
    t = (bn_beta - bn_mean * s + bias2).astype(np.float32)
    # activations are encoded as sign/2, so the conv yields y/2: scale by 2s.
    s2 = (2.0 * s).astype(np.float32)

    in_maps = []
    for c in range(N_CORES):
        in_maps.append(
            {
                "x": np.ascontiguousarray(x[c * IMG : (c + 1) * IMG]),
                "wq": wq,
                "b1": np.ascontiguousarray(bias1.reshape(-1)),
                "s": np.ascontiguousarray(s2.reshape(2, P)),
                "t": np.ascontiguousarray(t.reshape(2, P)),
            }
        )
    return in_maps


def _ensure_ntff_hook():
    """This container ships the NTFF profiling machinery but not the
    ``antenv.axon_hooks`` shim module bass_utils imports it through;
    synthesize it so trace=True can capture HW exec times."""
    import sys
    import types

    if "antenv.axon_hooks" in sys.modules:
        return
    import antenv
    from trn_agent_boot.trn_boot import _ntff_profile_via_ctypes

    hook = _ntff_profile_via_ctypes("/opt/axon/libaxon_pjrt.so")
    mod = types.ModuleType("antenv.axon_hooks")
    mod.get_axon_ntff_profile_hook = lambda: hook
    mod.set_axon_ntff_profile_hook = lambda h: None
    sys.modules["antenv.axon_hooks"] = mod
    antenv.axon_hooks = mod


def run(inputs: dict, trace: bool = False):
    """Run the SPMD kernel. Returns (out, exec_time_ns or None)."""
    nc = _get_nc()
    in_maps = _prep_inputs(**inputs)
    if trace:
        try:
            _ensure_ntff_hook()
        except Exception as e:  # degrade to untraced run
            print(f"ntff hook unavailable: {e}")
    res = run_bass_kernel_spmd(
        nc, in_maps, core_ids=list(range(N_CORES)), trace=trace
    )
    out = np.concatenate([r["out"] for r in res.results], axis=0)
    return out, res.exec_time_ns


def kernel(**inputs) -> np.ndarray:
    out, _ = run(inputs, trace=False)
    return out


# revision 45
# speedup vs baseline: 1.1598x; 1.0186x over previous
"""Trainium2 Bass kernel for nn_BasicBlock (binarized 3x3 conv + BN + ReLU).

Reference computation (NHWC, f32):
    a   = ste_sign(x + bias1)            # +-1, sign(0)=+1
    qk  = ste_sign(kernel)               # +-1
    y   = conv2d(a, qk, SAME, stride 1)  # (32,56,56,256)
    y   = (y - mean) * rsqrt(var+eps) + beta
    out = relu(y + bias2)

Strategy (v2, fp8 DoubleRow):
  - Data-parallel over batch: 8 cores x 4 images, no collectives.
  - Operands are exactly +-1, exact in fp8e4; fp32 PSUM accumulation keeps
    integer conv sums (|y| <= 2304) bit-exact. DoubleRow packs both
    128-channel Cin halves into one matmul at 2 MACs/cell/cycle.
  - Per core pipeline, per image:
      load 8-row groups [112 part, 4 px, 256ch] (4KB/partition descriptors)
      -> PE transpose f32 -> ScalarE Sign(x+bias1) -> fp8 +-1 into a
      zero-padded 58-wide channel-major image buffer
      -> conv: weights-stationary fp8 DoubleRow matmuls, psum [co,464px]
         (8 output rows per group, 9 taps accumulated, 2 Cout tiles)
      -> VectorE BN affine (y*s + t, per-partition scale/shift)
      -> PE transpose back to [px, co] per 2-row tile
      -> VectorE fused relu + PSUM evacuation into a per-image staging
         buffer -> 2 large NHWC stores per image via GPSIMD (SWDGE).
  - Host precomputes constants only: sign(kernel) in fp8 DoubleRow layout,
    folded BN scale/shift, and a 1-ulp nudge of x where fl(x+bias1)==0 so
    device Sign (sign(0)=0) matches ste_sign (sign(0)=+1).
"""

import numpy as np
import ml_dtypes

import concourse.bass as bass
import concourse.mybir as mybir
import concourse.tile as tile
from concourse import bacc
from concourse.bass_utils import run_bass_kernel_spmd
from concourse.masks import make_identity
from concourse.tile_rust import add_dep_helper

# Problem shape (hardcoded per contract).
B, H, W, CIN, COUT = 32, 56, 56, 256, 256
N_CORES = 8
IMG = B // N_CORES          # images per core
EPS = 1e-3

P = 128
WPAD = 58                   # padded row width (56 + 2)
ROWS = 59                   # 1 top pad + 56 real + 1 bottom pad + slack
AFREE = 3424                # ROWS*WPAD=3422 padded to %16 for DoubleRow APs
RP = H // 2                 # 28 row-pairs per image
G8 = H // 8                 # 7 eight-row conv groups per image
G4 = H // 4                 # 14 four-row input groups per image
NPX = 8 * WPAD              # 464 psum pixels per conv group

F32 = mybir.dt.float32
FP8 = mybir.dt.float8e4

AluOp = mybir.AluOpType


def _build_program():
    nc = bacc.Bacc(
        "TRN2",
        target_bir_lowering=False,
        debug=False,
        enable_asserts=False,
        num_devices=N_CORES,
    )

    x_ap = nc.dram_tensor("x", (IMG, H, W, CIN), F32, kind="ExternalInput").ap()
    w_ap = nc.dram_tensor("wq", (P, 9, 2, 2 * P), FP8, kind="ExternalInput").ap()
    b1_ap = nc.dram_tensor("b1", (2 * CIN,), F32, kind="ExternalInput").ap()
    s_ap = nc.dram_tensor("s", (2, P), F32, kind="ExternalInput").ap()
    t_ap = nc.dram_tensor("t", (2, P), F32, kind="ExternalInput").ap()
    out_ap = nc.dram_tensor("out", (IMG, H, W, COUT), F32, kind="ExternalOutput").ap()

    x_flat = x_ap.rearrange("b h w c -> b (h w) c")

    with tile.TileContext(nc) as tc:
        with (
            tc.tile_pool(name="const", bufs=1) as const_pool,
            tc.tile_pool(name="xin", bufs=4) as x_pool,
            tc.tile_pool(name="ybn", bufs=16) as y_pool,
            tc.tile_pool(name="pst", bufs=2, space="PSUM") as pst_pool,
            tc.tile_pool(name="pso", bufs=3, space="PSUM") as pso_pool,
            tc.tile_pool(name="psu", bufs=3, space="PSUM") as psu_pool,
        ):
            # Identity + activation-pad memsets first so the first image's
            # transposes/signs aren't queued behind the large constant DMAs.
            ident = const_pool.tile([P, P], F32)
            make_identity(nc, ident[:])
            identb = const_pool.tile([P, P], mybir.dt.bfloat16)
            make_identity(nc, identb[:])

            # Dummy ops to pull the one-time DVE/ACT microcode table loads
            # (~7us for the first tensor_scalar) off the critical path.
            warm = const_pool.tile([1, 8], F32)
            nc.vector.memset(warm[:], 0.0)
            nc.vector.tensor_scalar(
                warm[:], warm[:], 0.0, 0.5, op0=AluOp.is_ge, op1=AluOp.subtract
            )
            nc.scalar.activation(
                warm[:], warm[:], mybir.ActivationFunctionType.Identity,
                bias=0.0, scale=1.0,
            )
            acts = [
                const_pool.tile([P, 2, AFREE], FP8, name=f"act{i}") for i in range(2)
            ]
            nc.gpsimd.memset(acts[0][:], 0.0)
            nc.gpsimd.memset(acts[1][:], 0.0)

            b1_sb = const_pool.tile([P, 2 * CIN], F32)
            nc.sync.dma_start(b1_sb[:], b1_ap[None, :].to_broadcast((P, 2 * CIN)))
            # w/s/t are first needed by image 0's conv phase; issued inside
            # the b==0 iteration, after its input loads.
            w_sb = const_pool.tile([P, 9, 2, 2 * P], FP8)
            s_sb = const_pool.tile([P, 2], F32)
            t_sb = const_pool.tile([P, 2], F32)

            ubig = [
                const_pool.tile([P, RP, COUT], F32, name=f"ubig{i}") for i in range(2)
            ]

            def phase_a(b):
                # ---- load + binarize + transpose (4-row groups) ----
                # a = (fl(x + bias1) >= 0) - 0.5  in {-0.5, +0.5} bf16, exact
                # (the conv then yields y/2; the x2 is folded into BN scale).
                act = acts[b % 2]
                for g in range(G4):
                    xt = x_pool.tile([112, 2, CIN], F32, name="xt", tag="xt")
                    nc.sync.dma_start(
                        xt[:],
                        x_flat[b, 224 * g : 224 * (g + 1), :].rearrange(
                            "(p j) c -> p j c", p=112
                        ),
                    )
                    u = x_pool.tile([112, 2, CIN], F32, name="u", tag="u")
                    nc.vector.tensor_tensor(
                        u[:], xt[:],
                        b1_sb[:112].rearrange("p (j c) -> p j c", c=CIN),
                        AluOp.add,
                    )
                    a8 = x_pool.tile(
                        [112, 2, CIN], mybir.dt.bfloat16, name="a8", tag="a8"
                    )
                    nc.vector.tensor_scalar(
                        a8[:], u[:], 0.0, 0.5, op0=AluOp.is_ge, op1=AluOp.subtract
                    )
                    for j in range(2):
                        # both ci halves transposed into one psum tile, then
                        # one strided fp8 copy into the padded image buffer.
                        pt2 = pst_pool.tile([P, 224], mybir.dt.bfloat16)
                        for ci in range(2):
                            nc.tensor.matmul(
                                pt2[:, ci * 112 : (ci + 1) * 112],
                                a8[:, j, ci * P : (ci + 1) * P],
                                identb[:112, :112],
                                is_transpose=True,
                                start=(ci == 0),
                                stop=(ci == 1),
                            )
                        # px = 2p + j -> row r = px//56 (4 rows), col 2q+j.
                        # dest: padded rows 4g+1..4g+4, cols 2+j step 2.
                        base = (4 * g + 1) * WPAD + 2 + j
                        dest = (
                            act[:, :, base : base + 4 * WPAD]
                            .rearrange("p c (r w) -> p c r w", w=WPAD)[:, :, :, 0:56]
                            .rearrange("p c r (q x) -> p c r q x", x=2)[:, :, :, :, 0]
                        )
                        nc.scalar.copy(
                            dest,
                            pt2.rearrange("p (c r q) -> p c r q", c=2, q=28),
                        )

            phase_a(0)
            nc.sync.dma_start(w_sb[:], w_ap)
            nc.sync.dma_start(s_sb[:], s_ap.rearrange("t p -> p t"))
            nc.sync.dma_start(t_sb[:], t_ap.rearrange("t p -> p t"))

            for b in range(IMG):
                slot = b % 2
                act = acts[slot]
                ub = ubig[slot]

                # ---- conv (fp8 SwInterleave, weights stationary) + BN affine ----
                y_tiles = {}
                for co in range(2):
                    for m in range(G8):
                        pm = pso_pool.tile([P, NPX], F32, name="pm", tag="pm")
                        for tap in range(9):
                            dh, dw = tap // 3, tap % 3
                            rbase = (8 * m + dh) * WPAD + dw
                            nc.tensor.matmul(
                                pm[:],
                                w_sb[:, tap, co],
                                act[:, :, rbase : rbase + NPX],
                                start=(tap == 0),
                                stop=(tap == 8),
                                perf_mode=mybir.MatmulPerfMode.DoubleRowSwInterleave,
                            )
                        y = y_pool.tile([P, NPX], F32, name="y", tag="y")
                        # y = conv * scale + shift   (per-partition co consts)
                        nc.scalar.activation(
                            y[:], pm[:],
                            mybir.ActivationFunctionType.Identity,
                            bias=t_sb[:, co : co + 1],
                            scale=s_sb[:, co : co + 1],
                        )
                        y_tiles[(co, m)] = y

                # next image's input pipeline goes ahead of this image's
                # output phase so DVE/PE aren't blocked at the boundary.
                if b + 1 < IMG:
                    phase_a(b + 1)

                # ---- transpose back to [px, co], fused relu, stage, store ----
                ev = out_ap[b].rearrange("(k two) w c -> w two k c", two=2)
                for m in range(G8):
                    for r in range(4):
                        k = 4 * m + r
                        pu = psu_pool.tile([116, COUT], F32)
                        for co in range(2):
                            nc.tensor.matmul(
                                pu[:, co * P : (co + 1) * P],
                                y_tiles[(co, m)][:, 116 * r : 116 * r + 116],
                                ident[:, :P],
                                is_transpose=True,
                                start=(co == 0),
                                stop=(co == 1),
                            )
                        # partitions: 0 pad | 1..56 row 2k | 57,58 pad |
                        # 59..114 row 2k+1 | 115 pad
                        nc.vector.tensor_scalar(
                            ub[:116, k, :], pu[:], 0.0, None, op0=AluOp.max
                        )
                    # store the 8 finished rows (4 row-pairs) of this group,
                    # split across the two DGE paths so issue overlaps.
                    ksl = slice(4 * m, 4 * m + 4)
                    nc.gpsimd.dma_start(ev[:, 0, ksl], ub[1:57, ksl])
                    nc.gpsimd.dma_start(ev[:, 1, ksl], ub[59:115, ksl])

    nc.compile()
    return nc


_NC_CACHE = None


def _get_nc():
    global _NC_CACHE
    if _NC_CACHE is None:
        _NC_CACHE = _build_program()
    return _NC_CACHE


def _prep_inputs(x, bias1, kernel, bn_beta, bn_mean, bn_var, bias2):
    x = np.asarray(x, dtype=np.float32)
    bias1 = np.asarray(bias1, dtype=np.float32)
    kernel = np.asarray(kernel, dtype=np.float32)
    bn_beta = np.asarray(bn_beta, dtype=np.float32)
    bn_mean = np.asarray(bn_mean, dtype=np.float32)
    bn_var = np.asarray(bn_var, dtype=np.float32)
    bias2 = np.asarray(bias2, dtype=np.float32).reshape(-1)

    # Weights: ste_sign with sign(0)=+1, exact in fp8e4.
    # DoubleRowSwInterleave stationary layout per (tap, co_t):
    # [A127 B127 A126 B126 ... A0 B0] where A/B are the ci halves
    # (ci = o*128 + ki, matching the act buffer's [ki, ci_t, px] pairing)
    # and columns are stored co-reversed.
    wq = np.where(kernel >= 0, np.float32(1.0), np.float32(-1.0))
    wq = wq.reshape(9, 2, P, 2, P)[..., ::-1]       # [tap, o, ki, co_t, m]
    wq = wq.transpose(2, 0, 3, 4, 1)                # [ki, tap, co_t, m, o]
    wq = np.ascontiguousarray(wq).reshape(P, 9, 2, 2 * P)
    wq = wq.astype(ml_dtypes.float8_e4m3)

    s = (1.0 / np.sqrt(bn_var + np.float32(EPS))).astype(np.float32)
    t = (bn_beta - bn_mean * s + bias2).astype(np.float32)
    # activations are encoded as sign/2, so the conv yields y/2: scale by 2s.
    s2 = (2.0 * s).astype(np.float32)

    in_maps = []
    for c in range(N_CORES):
        in_maps.append(
            {
                "x": np.ascontiguousarray(x[c * IMG : (c + 1) * IMG]),
                "wq": wq,
                "b1": np.ascontiguousarray(np.tile(bias1.reshape(-1), 2)),
                "s": np.ascontiguousarray(s2.reshape(2, P)),
                "t": np.ascontiguousarray(t.reshape(2, P)),
            }
        )
    return in_maps


def _ensure_ntff_hook():
    """This container ships the NTFF profiling machinery but not the
    ``antenv.axon_hooks`` shim module bass_utils imports it through;
    synthesize it so trace=True can capture HW exec times."""
    import sys
    import types

    if "antenv.axon_hooks" in sys.modules:
        return
    import antenv
    from trn_agent_boot.trn_boot import _ntff_profile_via_ctypes

    hook = _ntff_profile_via_ctypes("/opt/axon/libaxon_pjrt.so")
    mod = types.ModuleType("antenv.axon_hooks")
    mod.get_axon_ntff_profile_hook = lambda: hook
    mod.set_axon_ntff_profile_hook = lambda h: None
    sys.modules["antenv.axon_hooks"] = mod
    antenv.axon_hooks = mod


def run(inputs: dict, trace: bool = False):
    """Run the SPMD kernel. Returns (out, exec_time_ns or None)."""
    nc = _get_nc()
    in_maps = _prep_inputs(**inputs)
    if trace:
        try:
            _ensure_ntff_hook()
        except Exception as e:  # degrade to untraced run
            print(f"ntff hook unavailable: {e}")
    res = run_bass_kernel_spmd(
        nc, in_maps, core_ids=list(range(N_CORES)), trace=trace
    )
    out = np.concatenate([r["out"] for r in res.results], axis=0)
    return out, res.exec_time_ns


def kernel(**inputs) -> np.ndarray:
    out, _ = run(inputs, trace=False)
    return out
